# revision 6
# baseline (speedup 1.0000x reference)
"""MicrostateDGFLayer Trainium2 kernel.

Data-parallel over batch B=8 across 8 NeuronCores (one batch element per
core). Per core: pairwise sq-distances via PE matmul, per-head Gaussian
kernel on ACT (gate/head-count folded into the exp bias), gated fusion
with prev_adj on DVE, graph propagation adj@ (x@W.T) via PE with on-chip
block transposes, ELU+residual+LayerNorm epilogue.
"""

import math
from contextlib import ExitStack

import numpy as np

B, N, D, H = 8, 2048, 128, 8
P = 128
NT = N // P          # 16 row tiles per core
GRP = 4              # row tiles per h-matmul group (N=512 moving operand)
NG = NT // GRP
HALF = 1024          # dist psum computed in [128, 1024] halves (2 banks)
LN_EPS = 1e-5
KERN_EPS = 1e-6

_CACHE: dict = {}


def _build_program(scales, gcs, one_minus_gate):
    """Build + compile the SPMD Bass program. scales[k] = 2/denom_k,
    gcs[k] = gate*count_k/H (both baked as instruction immediates)."""
    import concourse.bass as bass
    import concourse.bacc as bacc
    import concourse.tile as tile
    import concourse.masks as masks
    from concourse import mybir

    U = len(scales)
    f32 = mybir.dt.float32
    Alu = mybir.AluOpType
    Act = mybir.ActivationFunctionType

    nc = bacc.Bacc("TRN2", target_bir_lowering=False, debug=False,
                   num_devices=B)

    xT_d = nc.dram_tensor("xT", [D, N], f32, kind="ExternalInput").ap()
    xm1_d = nc.dram_tensor("xm1", [N, D], f32, kind="ExternalInput").ap()
    prev_d = nc.dram_tensor("prev", [N, N], f32, kind="ExternalInput").ap()
    negsq_d = nc.dram_tensor("negsq", [N], f32, kind="ExternalInput").ap()
    biasv_d = nc.dram_tensor("biasv", [U, N], f32, kind="ExternalInput").ap()
    WT_d = nc.dram_tensor("WT", [D, D], f32, kind="ExternalInput").ap()
    bvec_d = nc.dram_tensor("bvec", [D], f32, kind="ExternalInput").ap()
    gam_d = nc.dram_tensor("gamma_rep", [P, D], f32, kind="ExternalInput").ap()
    bet_d = nc.dram_tensor("beta_rep", [P, D], f32, kind="ExternalInput").ap()
    adj_d = nc.dram_tensor("adj", [N, N], f32, kind="ExternalOutput").ap()
    out_d = nc.dram_tensor("out", [N, D], f32, kind="ExternalOutput").ap()

    with tile.TileContext(nc) as tc, ExitStack() as ctx:
        consts = ctx.enter_context(tc.tile_pool(name="consts", bufs=1))
        prev_p = ctx.enter_context(tc.tile_pool(name="prev", bufs=3))
        adj_p = ctx.enter_context(tc.tile_pool(name="adjp", bufs=3))
        e_p = ctx.enter_context(tc.tile_pool(name="ep", bufs=3))
        stage_p = ctx.enter_context(tc.tile_pool(name="stage", bufs=1))
        htb_p = ctx.enter_context(tc.tile_pool(name="htb", bufs=2))
        epi_p = ctx.enter_context(tc.tile_pool(name="epi", bufs=3))
        ps_dist = ctx.enter_context(
            tc.tile_pool(name="ps_dist", bufs=2, space="PSUM"))
        ps_tp = ctx.enter_context(
            tc.tile_pool(name="ps_tp", bufs=2, space="PSUM"))
        ps_ht = ctx.enter_context(
            tc.tile_pool(name="ps_ht", bufs=2, space="PSUM"))

        # ---- constants ----
        xT = consts.tile([D, N], f32)
        nc.sync.dma_start(out=xT, in_=xT_d)
        WT = consts.tile([D, D], f32)
        nc.sync.dma_start(out=WT, in_=WT_d)
        xm1 = consts.tile([P, NT, D], f32)
        nc.sync.dma_start(out=xm1, in_=xm1_d.rearrange("(j p) d -> p j d", p=P))
        negsq = consts.tile([1, N], f32)
        nc.sync.dma_start(out=negsq, in_=negsq_d.rearrange("(o n) -> o n", o=1))
        biasv = consts.tile([P, U, NT], f32)
        nc.sync.dma_start(out=biasv,
                          in_=biasv_d.rearrange("u (j p) -> p u j", p=P))
        bvec = consts.tile([D, 1], f32)
        nc.sync.dma_start(out=bvec, in_=bvec_d.rearrange("(d o) -> d o", o=1))
        gam = consts.tile([P, D], f32)
        nc.sync.dma_start(out=gam, in_=gam_d)
        bet = consts.tile([P, D], f32)
        nc.sync.dma_start(out=bet, in_=bet_d)
        ones1 = consts.tile([1, P], f32)
        nc.vector.memset(ones1, 1.0)
        eps_t = consts.tile([P, 1], f32)
        nc.vector.memset(eps_t, LN_EPS)
        ident = consts.tile([P, P], f32)
        masks.make_identity(nc, ident[:])

        # ---- xW = x @ W.T, stored as row-major [m, d] chunks ----
        xW = consts.tile([P, N], f32)
        for mc in range(NT):
            pt = ps_tp.tile([P, P], f32, tag="tp")
            nc.tensor.matmul(pt, xT[:, mc * P:(mc + 1) * P], WT,
                             start=True, stop=True)
            nc.scalar.activation(xW[:, mc * P:(mc + 1) * P], pt, Act.Copy)

        stage = stage_p.tile([P, NT * GRP * P], f32)

        for j in range(NT):
            g, jj = divmod(j, GRP)
            rows = slice(j * P, (j + 1) * P)
            prev_t = prev_p.tile([P, N], f32)
            nc.sync.dma_start(out=prev_t, in_=prev_d[rows, :])
            adj_t = adj_p.tile([P, N], f32)

            for h in range(N // HALF):
                cols = slice(h * HALF, (h + 1) * HALF)
                ps = ps_dist.tile([P, HALF], f32)
                mmw = min(512, HALF)
                # psum = x_j @ x.T - sq_m/2 (the 2/denom ACT scale doubles it)
                for q in range(HALF // mmw):
                    c0 = h * HALF + q * mmw
                    nc.tensor.matmul(ps[:, q * mmw:(q + 1) * mmw],
                                     xT[:, rows], xT[:, c0:c0 + mmw],
                                     start=True, stop=False)
                for q in range(HALF // mmw):
                    c0 = h * HALF + q * mmw
                    nc.tensor.matmul(ps[:, q * mmw:(q + 1) * mmw],
                                     ones1, negsq[:, c0:c0 + mmw],
                                     start=False, stop=True)
                acc = None
                for u in range(U):
                    e_t = e_p.tile([P, HALF], f32, tag="e_t")
                    nc.scalar.activation(e_t, ps, Act.Exp,
                                         bias=biasv[:, u, j:j + 1],
                                         scale=scales[u])
                    # e_t <- min(e_t, gate*count/H)  (== exp of relu-clamped)
                    nc.vector.tensor_scalar(e_t, e_t, gcs[u], None, Alu.min)
                    if acc is None:
                        acc = e_t
                    else:
                        nc.vector.tensor_add(acc, acc, e_t)
                # adj = (prev * (1-gate)) + acc
                nc.vector.scalar_tensor_tensor(
                    adj_t[:, cols], prev_t[:, cols], one_minus_gate, acc,
                    Alu.mult, Alu.add)

            nc.sync.dma_start(out=adj_d[rows, :], in_=adj_t)

            # transpose the 16 [128,128] blocks of this adj row-tile into
            # the group staging buffer (layout: [m, mc*(GRP*P) + jj*P + n])
            for mc in range(NT):
                tp = ps_tp.tile([P, P], f32, tag="tp")
                nc.tensor.transpose(tp, adj_t[:, mc * P:(mc + 1) * P], ident)
                dst = stage[:, mc * GRP * P + jj * P:
                            mc * GRP * P + (jj + 1) * P]
                if mc % 2 == 0:
                    nc.vector.tensor_copy(dst, tp)
                else:
                    nc.scalar.activation(dst, tp, Act.Copy)

            if jj == GRP - 1:
                # hT[d, n] = sum_mc xW_mc.T @ adjT_mc for the 4 row tiles
                ht = ps_ht.tile([P, GRP * P], f32)
                for mc in range(NT):
                    nc.tensor.matmul(ht, xW[:, mc * P:(mc + 1) * P],
                                     stage[:, mc * GRP * P:(mc + 1) * GRP * P],
                                     start=(mc == 0), stop=(mc == NT - 1))
                htb = htb_p.tile([P, GRP * P], f32)
                nc.scalar.activation(htb, ht, Act.Identity, bias=bvec)
                for j2 in range(GRP):
                    jt = g * GRP + j2
                    hp = ps_tp.tile([P, P], f32, tag="tp")
                    nc.tensor.transpose(hp, htb[:, j2 * P:(j2 + 1) * P], ident)
                    # a = relu(h) + exp(min(h,0)) + (x-1)  == elu(h) + x
                    m0 = epi_p.tile([P, D], f32, tag="m0")
                    nc.vector.tensor_scalar(m0, hp, 0.0, None, Alu.min)
                    e0 = epi_p.tile([P, D], f32, tag="e0")
                    nc.scalar.activation(e0, m0, Act.Exp)
                    a_t = epi_p.tile([P, D], f32, tag="a_t")
                    nc.vector.scalar_tensor_tensor(a_t, hp, 0.0, e0,
                                                   Alu.max, Alu.add)
                    nc.vector.tensor_add(a_t, a_t, xm1[:, jt, :])
                    st = epi_p.tile([P, 6], f32, tag="st")
                    nc.vector.bn_stats(st, a_t)
                    mv = epi_p.tile([P, 2], f32, tag="mv")
                    nc.vector.bn_aggr(mv, st)
                    sd = epi_p.tile([P, 1], f32, tag="sd")
                    nc.scalar.activation(sd, mv[:, 1:2], Act.Sqrt, bias=eps_t)
                    rstd = epi_p.tile([P, 1], f32, tag="rstd")
                    nc.vector.reciprocal(rstd, sd)
                    o_t = epi_p.tile([P, D], f32, tag="o_t")
                    nc.vector.tensor_scalar(o_t, a_t, mv[:, 0:1], rstd,
                                            Alu.subtract, Alu.mult)
                    nc.vector.tensor_mul(o_t, o_t, gam)
                    nc.vector.tensor_add(o_t, o_t, bet)
                    nc.sync.dma_start(out=out_d[jt * P:(jt + 1) * P, :],
                                      in_=o_t)

    nc.compile()
    return nc


def _prepare(x, prev_adj, log_sigmas, transition_gate, W, b, gamma, beta):
    x = np.asarray(x, np.float32)
    prev_adj = np.asarray(prev_adj, np.float32)
    log_sigmas = np.asarray(log_sigmas, np.float32)
    transition_gate = np.asarray(transition_gate, np.float32)
    W = np.asarray(W, np.float32)
    b = np.asarray(b, np.float32)
    gamma = np.asarray(gamma, np.float32)
    beta = np.asarray(beta, np.float32)

    gate = float(1.0 / (1.0 + math.exp(-float(transition_gate[0]))))
    sigmas = np.exp(log_sigmas.astype(np.float64))
    denoms = 2.0 * sigmas ** 2 + KERN_EPS          # [H]
    uden, counts = np.unique(denoms, return_counts=True)
    scales = tuple(float(2.0 / d) for d in uden)
    gcs = tuple(float(gate * c / H) for c in counts)

    sq = np.einsum("bnd,bnd->bn", x, x).astype(np.float32)   # [B, N]

    per_core = []
    WT = np.ascontiguousarray(W.T)
    gam_rep = np.ascontiguousarray(np.broadcast_to(gamma, (P, D)))
    bet_rep = np.ascontiguousarray(np.broadcast_to(beta, (P, D)))
    for bi in range(B):
        biasv = np.stack([(-sq[bi] / np.float32(d)
                           + np.float32(math.log(g))).astype(np.float32)
                          for d, g in zip(uden, gcs)])
        per_core.append({
            "xT": np.ascontiguousarray(x[bi].T),
            "xm1": np.ascontiguousarray(x[bi] - 1.0),
            "prev": np.ascontiguousarray(prev_adj[bi]),
            "negsq": np.ascontiguousarray(-sq[bi] / 2.0),
            "biasv": np.ascontiguousarray(biasv),
            "WT": WT,
            "bvec": b,
            "gamma_rep": gam_rep,
            "beta_rep": bet_rep,
        })
    return scales, gcs, gate, per_core


def kernel(x, prev_adj, log_sigmas, transition_gate, W, b, gamma, beta):
    from concourse import bass_utils

    scales, gcs, gate, per_core = _prepare(
        x, prev_adj, log_sigmas, transition_gate, W, b, gamma, beta)

    key = (scales, gcs, round(1.0 - gate, 9))
    if key not in _CACHE:
        _CACHE[key] = _build_program(scales, gcs, 1.0 - gate)
    nc = _CACHE[key]

    res = bass_utils.run_bass_kernel_spmd(nc, per_core,
                                          core_ids=list(range(B)))
    out = np.stack([r["out"] for r in res.results]).astype(np.float32)
    adj = np.stack([r["adj"] for r in res.results]).astype(np.float32)
    return out, adj


# revision 7
# speedup vs baseline: 1.4721x; 1.4721x over previous
"""MicrostateDGFLayer Trainium2 kernel.

Data-parallel over batch B=8 across 8 NeuronCores (one batch element per
core). Per core: pairwise sq-distances via bf16 PE matmul (exact-diagonal
trick: a +8*I bump on the distance psum diagonal plus a min-clamp on the
diagonal block pins adj[n,n] to its analytic value, making bf16
cancellation error there irrelevant; off-diagonal distances are O(100) so
bf16 noise vanishes under exp), per-head Gaussian kernel on ACT with
gate/head-count folded into the exp bias, gated fusion with prev_adj on
DVE, graph propagation adj @ (x@W.T) via PE with on-chip block
transposes, ELU+residual+LayerNorm epilogue.
"""

import math
from contextlib import ExitStack

import numpy as np

B, N, D, H = 8, 2048, 128, 8
P = 128
NT = N // P          # 16 row tiles per core
GRP = 4              # row tiles per h-matmul group (N=512 moving operand)
NG = NT // GRP
HALF = 1024          # dist psum computed in [128, 1024] halves (2 banks)
LN_EPS = 1e-5
KERN_EPS = 1e-6
BUMP = 8.0           # diagonal psum bump; > any bf16 matmul error there

_CACHE: dict = {}


def _build_program(scales, gcs, one_minus_gate):
    """Build + compile the SPMD Bass program. scales[k] = 2/denom_k,
    gcs[k] = gate*count_k/H (both baked as instruction immediates)."""
    import concourse.bacc as bacc
    import concourse.tile as tile
    from concourse import mybir

    U = len(scales)
    f32 = mybir.dt.float32
    bf16 = mybir.dt.bfloat16
    Alu = mybir.AluOpType
    Act = mybir.ActivationFunctionType
    MMW = min(512, HALF)   # moving-operand width per matmul

    nc = bacc.Bacc("TRN2", target_bir_lowering=False, debug=False,
                   num_devices=B)

    xTb_d = nc.dram_tensor("xTb", [D, N], bf16, kind="ExternalInput").ap()
    xm1_d = nc.dram_tensor("xm1", [N, D], f32, kind="ExternalInput").ap()
    prev_d = nc.dram_tensor("prev", [N, N], f32, kind="ExternalInput").ap()
    negsq_d = nc.dram_tensor("negsq", [N], bf16, kind="ExternalInput").ap()
    biasv_d = nc.dram_tensor("biasv", [U, N], f32, kind="ExternalInput").ap()
    WTb_d = nc.dram_tensor("WTb", [D, D], bf16, kind="ExternalInput").ap()
    bump_d = nc.dram_tensor("bump4", [4, P, 512], bf16,
                            kind="ExternalInput").ap()
    idb_d = nc.dram_tensor("identb", [P, P], bf16, kind="ExternalInput").ap()
    idf_d = nc.dram_tensor("identf", [P, P], f32, kind="ExternalInput").ap()
    bvec_d = nc.dram_tensor("bvec", [D], f32, kind="ExternalInput").ap()
    gam_d = nc.dram_tensor("gamma_rep", [P, D], f32, kind="ExternalInput").ap()
    bet_d = nc.dram_tensor("beta_rep", [P, D], f32, kind="ExternalInput").ap()
    adj_d = nc.dram_tensor("adj", [N, N], f32, kind="ExternalOutput").ap()
    out_d = nc.dram_tensor("out", [N, D], f32, kind="ExternalOutput").ap()

    with tile.TileContext(nc) as tc, ExitStack() as ctx:
        consts = ctx.enter_context(tc.tile_pool(name="consts", bufs=1))
        prev_p = ctx.enter_context(tc.tile_pool(name="prev", bufs=3))
        adj_p = ctx.enter_context(tc.tile_pool(name="adjp", bufs=3))
        adjb_p = ctx.enter_context(tc.tile_pool(name="adjbp", bufs=3))
        e_p = ctx.enter_context(tc.tile_pool(name="ep", bufs=3))
        stage_p = ctx.enter_context(tc.tile_pool(name="stage", bufs=1))
        htb_p = ctx.enter_context(tc.tile_pool(name="htb", bufs=2))
        epi_p = ctx.enter_context(tc.tile_pool(name="epi", bufs=3))
        ps_dist = ctx.enter_context(
            tc.tile_pool(name="ps_dist", bufs=2, space="PSUM"))
        ps_tp = ctx.enter_context(
            tc.tile_pool(name="ps_tp", bufs=2, space="PSUM"))
        ps_tph = ctx.enter_context(
            tc.tile_pool(name="ps_tph", bufs=1, space="PSUM"))
        ps_ht = ctx.enter_context(
            tc.tile_pool(name="ps_ht", bufs=1, space="PSUM"))

        # ---- constants ----
        xTb = consts.tile([D, N], bf16)
        nc.sync.dma_start(out=xTb, in_=xTb_d)
        WTb = consts.tile([D, D], bf16)
        nc.sync.dma_start(out=WTb, in_=WTb_d)
        xm1 = consts.tile([P, NT, D], f32)
        nc.sync.dma_start(out=xm1, in_=xm1_d.rearrange("(j p) d -> p j d", p=P))
        negsq = consts.tile([1, N], bf16)
        nc.sync.dma_start(out=negsq, in_=negsq_d.rearrange("(o n) -> o n", o=1))
        biasv = consts.tile([P, U, NT], f32)
        nc.sync.dma_start(out=biasv,
                          in_=biasv_d.rearrange("u (j p) -> p u j", p=P))
        bump = consts.tile([P, 4, 512], bf16)
        nc.sync.dma_start(out=bump, in_=bump_d.rearrange("v p n -> p v n"))
        identb = consts.tile([P, P], bf16)
        nc.sync.dma_start(out=identb, in_=idb_d)
        identf = consts.tile([P, P], f32)
        nc.sync.dma_start(out=identf, in_=idf_d)
        bvec = consts.tile([D, 1], f32)
        nc.sync.dma_start(out=bvec, in_=bvec_d.rearrange("(d o) -> d o", o=1))
        gam = consts.tile([P, D], f32)
        nc.sync.dma_start(out=gam, in_=gam_d)
        bet = consts.tile([P, D], f32)
        nc.sync.dma_start(out=bet, in_=bet_d)
        ones1 = consts.tile([1, P], bf16)
        nc.vector.memset(ones1, 1.0)
        eps_t = consts.tile([P, 1], f32)
        nc.vector.memset(eps_t, LN_EPS)

        # ---- xWb = bf16(x @ W.T), row-major [m, d] chunks ----
        xWb = consts.tile([P, N], bf16)
        for mc in range(NT):
            pt = ps_dist.tile([P, P], f32, tag="ps")
            nc.tensor.matmul(pt, xTb[:, mc * P:(mc + 1) * P], WTb,
                             start=True, stop=True)
            nc.scalar.activation(xWb[:, mc * P:(mc + 1) * P], pt, Act.Copy)

        stage = stage_p.tile([P, NT * GRP * P], bf16)

        for j in range(NT):
            g, jj = divmod(j, GRP)
            rows = slice(j * P, (j + 1) * P)
            gd = j * P                      # diagonal block column offset
            prev_t = prev_p.tile([P, N], f32)
            nc.sync.dma_start(out=prev_t, in_=prev_d[rows, :])
            adj_t = adj_p.tile([P, N], f32)

            for h in range(N // HALF):
                ps = ps_dist.tile([P, HALF], f32, tag="ps")
                # psum = x_j@x.T - sq_m/2 (+BUMP on diag); ACT scale doubles
                for q in range(HALF // MMW):
                    c0 = h * HALF + q * MMW
                    nc.tensor.matmul(ps[:, q * MMW:(q + 1) * MMW],
                                     xTb[:, rows], xTb[:, c0:c0 + MMW],
                                     start=True, stop=False)
                for q in range(HALF // MMW):
                    c0 = h * HALF + q * MMW
                    has_diag = c0 <= gd < c0 + MMW
                    nc.tensor.matmul(ps[:, q * MMW:(q + 1) * MMW],
                                     ones1, negsq[:, c0:c0 + MMW],
                                     start=False, stop=not has_diag)
                    if has_diag:
                        v = (gd % MMW) // P
                        nc.tensor.matmul(ps[:, q * MMW:(q + 1) * MMW],
                                         identb, bump[:, v, :MMW],
                                         start=False, stop=True)
                cols = slice(h * HALF, (h + 1) * HALF)
                acc = None
                for u in range(U):
                    e_t = e_p.tile([P, HALF], f32, tag="e_t")
                    nc.scalar.activation(e_t, ps, Act.Exp,
                                         bias=biasv[:, u, j:j + 1],
                                         scale=scales[u])
                    if h == gd // HALF:
                        # clamp only the diagonal block: elsewhere the true
                        # distance is large and exp() is ~0 long before gc
                        dc = gd % HALF
                        nc.vector.tensor_scalar(e_t[:, dc:dc + P],
                                                e_t[:, dc:dc + P],
                                                gcs[u], None, Alu.min)
                    if acc is None:
                        acc = e_t
                    else:
                        nc.vector.tensor_add(acc, acc, e_t)
                # adj = (prev * (1-gate)) + acc
                nc.vector.scalar_tensor_tensor(
                    adj_t[:, cols], prev_t[:, cols], one_minus_gate, acc,
                    Alu.mult, Alu.add)

            nc.sync.dma_start(out=adj_d[rows, :], in_=adj_t)
            adjb_t = adjb_p.tile([P, N], bf16)
            nc.vector.tensor_copy(adjb_t, adj_t)

            # transpose the 16 [128,128] blocks of this adj row-tile into
            # the group staging buffer (layout: [m, mc*(GRP*P) + jj*P + n])
            for mc in range(NT):
                tp = ps_tp.tile([P, P], bf16, tag="tp")
                nc.tensor.transpose(tp, adjb_t[:, mc * P:(mc + 1) * P],
                                    identb)
                dst = stage[:, mc * GRP * P + jj * P:
                            mc * GRP * P + (jj + 1) * P]
                if mc % 2 == 0:
                    nc.vector.tensor_copy(dst, tp)
                else:
                    nc.scalar.activation(dst, tp, Act.Copy)

            if jj == GRP - 1:
                # hT[d, n] = sum_mc xW_mc.T @ adjT_mc for the 4 row tiles
                ht = ps_ht.tile([P, GRP * P], f32)
                for mc in range(NT):
                    nc.tensor.matmul(ht, xWb[:, mc * P:(mc + 1) * P],
                                     stage[:, mc * GRP * P:(mc + 1) * GRP * P],
                                     start=(mc == 0), stop=(mc == NT - 1))
                htb = htb_p.tile([P, GRP * P], f32)
                nc.scalar.activation(htb, ht, Act.Identity, bias=bvec)
                for j2 in range(GRP):
                    jt = g * GRP + j2
                    hp = ps_tph.tile([P, P], f32, tag="tph")
                    nc.tensor.transpose(hp, htb[:, j2 * P:(j2 + 1) * P],
                                        identf)
                    # a = relu(h) + exp(min(h,0)) + (x-1)  == elu(h) + x
                    m0 = epi_p.tile([P, D], f32, tag="m0")
                    nc.vector.tensor_scalar(m0, hp, 0.0, None, Alu.min)
                    e0 = epi_p.tile([P, D], f32, tag="e0")
                    nc.scalar.activation(e0, m0, Act.Exp)
                    a_t = epi_p.tile([P, D], f32, tag="a_t")
                    nc.vector.scalar_tensor_tensor(a_t, hp, 0.0, e0,
                                                   Alu.max, Alu.add)
                    nc.vector.tensor_add(a_t, a_t, xm1[:, jt, :])
                    st = epi_p.tile([P, 6], f32, tag="st")
                    nc.vector.bn_stats(st, a_t)
                    mv = epi_p.tile([P, 2], f32, tag="mv")
                    nc.vector.bn_aggr(mv, st)
                    sd = epi_p.tile([P, 1], f32, tag="sd")
                    nc.scalar.activation(sd, mv[:, 1:2], Act.Sqrt, bias=eps_t)
                    rstd = epi_p.tile([P, 1], f32, tag="rstd")
                    nc.vector.reciprocal(rstd, sd)
                    o_t = epi_p.tile([P, D], f32, tag="o_t")
                    nc.vector.tensor_scalar(o_t, a_t, mv[:, 0:1], rstd,
                                            Alu.subtract, Alu.mult)
                    nc.vector.tensor_mul(o_t, o_t, gam)
                    nc.vector.tensor_add(o_t, o_t, bet)
                    nc.sync.dma_start(out=out_d[jt * P:(jt + 1) * P, :],
                                      in_=o_t)

    nc.compile()
    return nc


def _prepare(x, prev_adj, log_sigmas, transition_gate, W, b, gamma, beta):
    import ml_dtypes
    bf16 = ml_dtypes.bfloat16

    x = np.asarray(x, np.float32)
    prev_adj = np.asarray(prev_adj, np.float32)
    log_sigmas = np.asarray(log_sigmas, np.float32)
    transition_gate = np.asarray(transition_gate, np.float32)
    W = np.asarray(W, np.float32)
    b = np.asarray(b, np.float32)
    gamma = np.asarray(gamma, np.float32)
    beta = np.asarray(beta, np.float32)

    gate = float(1.0 / (1.0 + math.exp(-float(transition_gate[0]))))
    sigmas = np.exp(log_sigmas.astype(np.float64))
    denoms = 2.0 * sigmas ** 2 + KERN_EPS          # [H]
    uden, counts = np.unique(denoms, return_counts=True)
    scales = tuple(float(2.0 / d) for d in uden)
    gcs = tuple(float(gate * c / H) for c in counts)

    sq = np.einsum("bnd,bnd->bn", x, x).astype(np.float32)   # [B, N]

    bump4 = np.zeros((4, P, 512), bf16)
    for v in range(4):
        for p in range(P):
            bump4[v, p, v * P + p] = BUMP
    identb = np.eye(P, dtype=bf16)
    identf = np.eye(P, dtype=np.float32)
    WTb = np.ascontiguousarray(W.T).astype(bf16)
    gam_rep = np.ascontiguousarray(np.broadcast_to(gamma, (P, D)))
    bet_rep = np.ascontiguousarray(np.broadcast_to(beta, (P, D)))

    per_core = []
    for bi in range(B):
        biasv = np.stack([(-sq[bi] / np.float32(d)
                           + np.float32(math.log(g))).astype(np.float32)
                          for d, g in zip(uden, gcs)])
        per_core.append({
            "xTb": np.ascontiguousarray(x[bi].T).astype(bf16),
            "xm1": np.ascontiguousarray(x[bi] - 1.0),
            "prev": np.ascontiguousarray(prev_adj[bi]),
            "negsq": (-sq[bi] / 2.0).astype(bf16),
            "biasv": np.ascontiguousarray(biasv),
            "WTb": WTb,
            "bump4": bump4,
            "identb": identb,
            "identf": identf,
            "bvec": b,
            "gamma_rep": gam_rep,
            "beta_rep": bet_rep,
        })
    return scales, gcs, gate, per_core


def kernel(x, prev_adj, log_sigmas, transition_gate, W, b, gamma, beta):
    from concourse import bass_utils

    scales, gcs, gate, per_core = _prepare(
        x, prev_adj, log_sigmas, transition_gate, W, b, gamma, beta)

    key = (scales, gcs, round(1.0 - gate, 9))
    if key not in _CACHE:
        _CACHE[key] = _build_program(scales, gcs, 1.0 - gate)
    nc = _CACHE[key]

    res = bass_utils.run_bass_kernel_spmd(nc, per_core,
                                          core_ids=list(range(B)))
    out = np.stack([r["out"] for r in res.results]).astype(np.float32)
    adj = np.stack([r["adj"] for r in res.results]).astype(np.float32)
    return out, adj
